# revision 2
# baseline (speedup 1.0000x reference)
"""Trainium2 Bass kernel for nn_AttentionHead_5583457485447 (sparse_attention).

Reference computation (per batch b):
    q = X @ Wq; k = X @ Wk                      # [N, DK]
    s = relu((q @ k.T) / sqrt(DK)) * M_mask     # [N, N]
    out = s @ Z @ Wv                            # [N, DV]

Strategy (8 NeuronCores, data-parallel over batch B=8, one batch per core):
  - Fold 1/sqrt(DK) into Wv (relu is positively homogeneous, rest is linear).
  - Mask quantized to uint8 (m8 = round(m*255)); the 1/255 is folded into
    Wv as well.  Halves mask HBM traffic; adds ~2e-3 rel error (budget 2e-2).
  - ZW = Z @ (Wv/(8*255)) computed on device; the v=256 contraction sliver
    (rank-1 term z256 (x) wv256) is NOT done with K=1 matmuls: z256/255 is
    appended as a 258th column to each zw tile, so the big C matmul
    accumulates u[n] = masked8 @ z256' for free, and a fused DVE
    scalar_tensor_tensor adds u[p]*wvb + psum during the PSUM->SBUF copy.
  - q/k projections run concurrently in PE column groups 0:64 / 64:128 of
    one pass (tile_position=(0,64) for k), giving stacked [qT; kT] tiles;
    the swapped [kT; qT] layout is produced by SBUF->SBUF DMA so the score
    matmuls can row-pack two K=64 m-chunks per PE pass (rows 0:64 / 64:128).
  - Scores computed directly in transposed [m, n] layout (lhsT = kT,
    rhs = qT): already the lhsT layout the second matmul needs.
  - relu+mask fused: DVE scalar_tensor_tensor max(s,0)*m8 from PSUM for most
    tiles; ACT relu + GpSimd multiply for the rest (engine balance).
  - Quarter-granularity software pipeline (n in 4 chunks of 512): C(q)
    overlaps B(q+2) elementwise + scores, masks stream in per quarter.
"""

import json
import os
import sys

import numpy as np

B, N, D, DK = 8, 2048, 256, 64
DV = D + 1  # 257
NT = N // 128  # 16 tiles along n and along m
PW = 512  # quarter width = scores matmul moving width
NQ = N // PW  # 4 quarters
QT = PW // 128  # 4 n-tiles per quarter

LAST_EXEC_NS = None
_CACHE = {}


# --------------------------------------------------------------------------
# Patch: this container's walrus build rejects instructions carrying more
# than one semaphore wait. Split excess waits onto same-engine NOPs at the
# serialized-BIR level (generic, covers Tile's drains and compute ops).
# --------------------------------------------------------------------------
def _split_waits_in_bir(bir_json: bytes) -> bytes:
    bir = json.loads(bir_json)
    changed = False
    for fn in bir.get("functions", []):
        for bb in fn.get("blocks", []):
            insts = bb.get("instructions", [])
            out = []
            for inst in insts:
                si = inst.get("sync_info")
                ow = (si or {}).get("on_wait") or []
                if len(ow) > 1:
                    changed = True
                    for i, w in enumerate(ow[:-1]):
                        out.append({
                            "debug": inst.get("debug", 0),
                            "engine": inst["engine"],
                            "ins": [],
                            "name": f"{inst['name']}-ws{i}",
                            "opcode": "NoOp",
                            "outs": [],
                            "sync_info": {"on_update": [], "on_wait": [w]},
                            "text_hint": "wait_split",
                        })
                    si["on_wait"] = [ow[-1]]
                out.append(inst)
            bb["instructions"] = out
    return json.dumps(bir).encode() if changed else bir_json


def _apply_bir_patch():
    import concourse.bass_utils as bass_utils
    import concourse.bass2jax as bass2jax

    orig = bass_utils.compile_bir_kernel
    if getattr(orig, "_wait_split_wrapped", False):
        return

    def wrapped(bir_json, tmpdir, neff_name="file.neff"):
        if isinstance(bir_json, str):
            bir_json = bir_json.encode()
        return orig(_split_waits_in_bir(bir_json), tmpdir, neff_name=neff_name)

    wrapped._wait_split_wrapped = True
    bass_utils.compile_bir_kernel = wrapped
    bass2jax.compile_bir_kernel = wrapped


# --------------------------------------------------------------------------
# Optional NTFF profiling hook for axon (exec-time measurement).
# Only used when KERNEL_TRACE=1; missing in this image's antenv.
# --------------------------------------------------------------------------
def _install_profile_shim():
    import types, ctypes, contextlib

    if "antenv.axon_hooks" in sys.modules:
        return
    so_path = "/opt/axon/libaxon_pjrt.so"
    if not os.path.exists(so_path):
        return
    lib = ctypes.CDLL(so_path)
    if not hasattr(lib, "axon_start_nrt_profile"):
        return
    lib.axon_start_nrt_profile.argtypes = [ctypes.POINTER(ctypes.c_int64), ctypes.c_size_t]
    lib.axon_start_nrt_profile.restype = ctypes.c_int64
    lib.axon_stop_nrt_profile.argtypes = [ctypes.c_char_p]
    lib.axon_stop_nrt_profile.restype = ctypes.c_int64

    @contextlib.contextmanager
    def _hook(output_dir, device_ids):
        import jax

        jax.devices()
        if device_ids:
            ids = (ctypes.c_int64 * len(device_ids))(*device_ids)
            rc = lib.axon_start_nrt_profile(ids, len(device_ids))
        else:
            rc = lib.axon_start_nrt_profile(None, 0)
        if rc != 0:
            raise RuntimeError(f"axon_start_nrt_profile rc={rc}")
        try:
            yield
        finally:
            n = lib.axon_stop_nrt_profile(str(output_dir).encode())
            print(f"profile: {n} file(s) written to {output_dir}", file=sys.stderr)

    mod = types.ModuleType("antenv.axon_hooks")
    mod.get_axon_ntff_profile_hook = lambda: _hook
    sys.modules["antenv.axon_hooks"] = mod


# --------------------------------------------------------------------------
# Device program (identical for all 8 cores; one batch per core)
# --------------------------------------------------------------------------
def _build_nc():
    import concourse.bass as bass
    import concourse.mybir as mybir
    import concourse.tile as tile

    f32 = mybir.dt.float32
    bf16 = mybir.dt.bfloat16
    u8 = mybir.dt.uint8
    Alu = mybir.AluOpType
    Act = mybir.ActivationFunctionType

    nc = bass.Bass("TRN2", debug=False)

    d_m8T = nc.dram_tensor("m8T", [N, N], u8, kind="ExternalInput")
    d_XT = nc.dram_tensor("XT", [D, N], bf16, kind="ExternalInput")
    d_ZT = nc.dram_tensor("ZT", [D, N], bf16, kind="ExternalInput")  # rows 0:256 of Z^T
    d_z256 = nc.dram_tensor("z256", [128, NT], bf16, kind="ExternalInput")
    d_wvb = nc.dram_tensor("wvb", [128, DV], bf16, kind="ExternalInput")
    d_Wq = nc.dram_tensor("Wq", [D, DK], bf16, kind="ExternalInput")
    d_Wk = nc.dram_tensor("Wk", [D, DK], bf16, kind="ExternalInput")
    d_Wv2 = nc.dram_tensor("Wv2", [D, DV], bf16, kind="ExternalInput")
    d_out = nc.dram_tensor("out", [N, DV], f32, kind="ExternalOutput")

    with tile.TileContext(nc) as tc:
        with (
            tc.tile_pool(name="prep", bufs=1) as prep,       # XT/ZT staging
            tc.tile_pool(name="wts", bufs=1) as wts,         # weights + QK tiles
            tc.tile_pool(name="maskp", bufs=1) as maskp,     # all mask tiles resident
            tc.tile_pool(name="maskedp", bufs=1) as maskedp, # all masked tiles resident
            tc.tile_pool(name="zwp", bufs=1) as zwp,         # bf16 ZW tiles (258 wide)
            tc.tile_pool(name="outp", bufs=3) as outp,       # out staging
            tc.tile_pool(name="rlp", bufs=4) as rlp,         # relu staging (ACT path)
            tc.tile_pool(name="psS", bufs=5, space="PSUM") as psS,   # proj/zw/scores
            tc.tile_pool(name="psC", bufs=3, space="PSUM") as psC,   # C groups
        ):
            # ---- input DMAs, in priority order ----
            wq_sb = [wts.tile([128, DK], bf16, tag=f"wq{c}", name=f"wq{c}") for c in range(2)]
            wk_sb = [wts.tile([128, DK], bf16, tag=f"wk{c}", name=f"wk{c}") for c in range(2)]
            for c in range(2):
                nc.gpsimd.dma_start(wq_sb[c][:], d_Wq.ap()[c * 128:(c + 1) * 128, :])
                nc.gpsimd.dma_start(wk_sb[c][:], d_Wk.ap()[c * 128:(c + 1) * 128, :])
            xt_sb = [[prep.tile([128, PW], bf16, tag=f"xt{c}_{g}", name=f"xt{c}_{g}")
                      for g in range(NQ)] for c in range(2)]
            for g in range(NQ):
                for c in range(2):
                    nc.sync.dma_start(
                        xt_sb[c][g][:],
                        d_XT.ap()[c * 128:(c + 1) * 128, g * PW:(g + 1) * PW],
                    )
            wv2_sb = [wts.tile([128, DV], bf16, tag=f"wv{i}", name=f"wv{i}") for i in range(2)]
            for i in range(2):
                nc.scalar.dma_start(wv2_sb[i][:], d_Wv2.ap()[i * 128:(i + 1) * 128, :])
            z256_sb = wts.tile([128, NT], bf16, tag="z256", name="z256")
            nc.scalar.dma_start(z256_sb[:], d_z256.ap()[:, :])
            wvb_sb = wts.tile([128, DV], bf16, tag="wvb", name="wvb")
            nc.scalar.dma_start(wvb_sb[:], d_wvb.ap()[:, :])
            zt_sb = [prep.tile([128, N], bf16, tag=f"zt{i}", name=f"zt{i}") for i in range(2)]
            for i in range(2):
                nc.scalar.dma_start(zt_sb[i][:], d_ZT.ap()[i * 128:(i + 1) * 128, :])

            # mask tiles: [128, PW] u8, all resident; emit q0,q1 first
            mk = {}
            for q in range(NQ):
                for mt in range(NT):
                    t = maskp.tile([128, PW], u8, tag=f"mk{q}_{mt}", name=f"mk{q}_{mt}")
                    mk[(q, mt)] = t
                    nc.sync.dma_start(
                        t[:],
                        d_m8T.ap()[mt * 128:(mt + 1) * 128, q * PW:(q + 1) * PW],
                    )

            # ---- projections: q into PE cols 0:64, k into cols 64:128 ----
            # psQK = [qT_g (64 rows); kT_g (64 rows)] per column chunk g.
            qk_a = [wts.tile([128, PW], bf16, tag=f"qka{g}", name=f"qka{g}") for g in range(NQ)]
            qk_b = [wts.tile([128, PW], bf16, tag=f"qkb{g}", name=f"qkb{g}") for g in range(NQ)]
            for g in range(NQ):
                ps = psS.tile([128, PW], f32, tag="psS", name=f"psqk{g}")
                for c in range(2):
                    nc.tensor.matmul(
                        ps[0:DK, :], wq_sb[c][:], xt_sb[c][g][:],
                        start=(c == 0), stop=(c == 1),
                    )
                for c in range(2):
                    nc.tensor.matmul(
                        ps[DK:128, :], wk_sb[c][:], xt_sb[c][g][:],
                        start=(c == 0), stop=(c == 1),
                        tile_position=(0, DK),
                    )
                nc.scalar.activation(qk_a[g][:], ps[:], Act.Copy)
                # swapped layout for the row-packed score matmuls
                nc.scalar.dma_start(qk_b[g][DK:128, :], qk_a[g][0:DK, :])
                nc.scalar.dma_start(qk_b[g][0:DK, :], qk_a[g][DK:128, :])

            # ---- ZW = Z[:, :256] @ Wv2, with z256/255 appended as col 257 ----
            zw_sb = []
            for mt in range(NT):
                ps = psS.tile([128, DV], f32, tag="psS", name=f"pszw{mt}")
                for i in range(2):
                    nc.tensor.matmul(
                        ps[:],
                        zt_sb[i][:, mt * 128:(mt + 1) * 128],
                        wv2_sb[i][:],
                        start=(i == 0), stop=(i == 1),
                    )
                zw = zwp.tile([128, DV + 1], bf16, tag=f"zw{mt}", name=f"zw{mt}")
                nc.scalar.activation(zw[:, 0:DV], ps[:], Act.Copy)
                nc.vector.tensor_copy(zw[:, DV:DV + 1], z256_sb[:, mt:mt + 1])
                zw_sb.append(zw)

            # ---- B(q): scores + relu*mask for one n-quarter ----
            masked_sb = {}
            ew = 0  # elementwise rotation counter

            def emit_b_pair(q, pr):
                nonlocal ew
                mts = (2 * pr, 2 * pr + 1)
                for j, mt in enumerate(mts):
                    gk, kcol = divmod(mt * 128, PW)
                    ro = DK * j
                    # j=0: lhsT = kT from qk_b rows 0:64; rhs = qT from qk_a rows 0:64
                    # j=1: lhsT = kT from qk_a rows 64:128; rhs = qT from qk_b rows 64:128
                    lhsT = (qk_b if j == 0 else qk_a)[gk][ro:ro + DK, kcol:kcol + 128]
                    rhs = (qk_a if j == 0 else qk_b)[q][ro:ro + DK, :]
                    ps = psS.tile([128, PW], f32, tag="psS", name=f"pss{q}_{mt}")
                    nc.tensor.matmul(ps[:], lhsT, rhs, start=True, stop=True)
                    md = maskedp.tile([128, PW], bf16, tag=f"md{q}_{mt}", name=f"md{q}_{mt}")
                    masked_sb[(q, mt)] = md
                    if ew % 8 in (2, 5, 7):
                        rl = rlp.tile([128, PW], bf16, tag="rl", name=f"rl{q}_{mt}")
                        nc.scalar.activation(rl[:], ps[:], Act.Relu)
                        nc.gpsimd.tensor_mul(md[:], rl[:], mk[(q, mt)][:])
                    else:
                        nc.vector.scalar_tensor_tensor(
                            md[:], ps[:], 0.0, mk[(q, mt)][:], Alu.max, Alu.mult,
                        )
                    ew += 1

            # ---- C(q, nt): out n-tile = sum_mt masked.T @ zw ----
            def emit_c_group(q, i):
                nt = q * QT + i
                ps = psC.tile([128, DV + 1], f32, tag="psC", name=f"psc{nt}")
                for mt in range(NT):
                    nc.tensor.matmul(
                        ps[:],
                        masked_sb[(q, mt)][:, i * 128:(i + 1) * 128],
                        zw_sb[mt][:],
                        start=(mt == 0), stop=(mt == NT - 1),
                    )
                ot = outp.tile([128, DV], f32, tag="out", name=f"ot{nt}")
                # out = wvb * u + main   (u = psum col 257)
                nc.vector.scalar_tensor_tensor(
                    ot[:], wvb_sb[:], ps[:, DV:DV + 1], ps[:, 0:DV],
                    Alu.mult, Alu.add,
                )
                nc.sync.dma_start(
                    d_out.ap()[nt * 128:(nt + 1) * 128, :], ot[:]
                )

            # ---- software pipeline over quarters ----
            for pr in range(NT // 2):
                emit_b_pair(0, pr)
            for pr in range(NT // 2):
                emit_b_pair(1, pr)
            for q in (2, 3):
                for pr in range(NT // 2):
                    emit_b_pair(q, pr)
                    if pr % 2 == 1:
                        emit_c_group(q - 2, pr // 2)
            for q in (2, 3):
                for i in range(QT):
                    emit_c_group(q, i)

    return nc


def kernel(Z_l, X_l, M_mask, Wq, Wk, Wv):
    global LAST_EXEC_NS
    _apply_bir_patch()

    trace = os.environ.get("KERNEL_TRACE", "0") == "1"
    if trace:
        _install_profile_shim()

    from concourse.bass_utils import run_bass_kernel_spmd

    Z_l = np.asarray(Z_l, dtype=np.float32)
    X_l = np.asarray(X_l, dtype=np.float32)
    M_mask = np.asarray(M_mask, dtype=np.float32)
    Wq = np.asarray(Wq, dtype=np.float32)
    Wk = np.asarray(Wk, dtype=np.float32)
    Wv = np.asarray(Wv, dtype=np.float32)

    import ml_dtypes
    bf = ml_dtypes.bfloat16

    # Host-side layout prep (transpose + casts) + scale folds.
    scale = np.float32(1.0 / (np.sqrt(np.float32(DK)) * 255.0))
    XT = np.ascontiguousarray(X_l.transpose(0, 2, 1)).astype(bf)          # [B, D, N]
    ZT = np.ascontiguousarray(Z_l[:, :, :D].transpose(0, 2, 1)).astype(bf)  # [B, 256, N]
    M8T = np.ascontiguousarray(
        np.clip(np.round(M_mask * 255.0), 0, 255).astype(np.uint8).transpose(0, 2, 1)
    )                                                                      # [B, N(m), N(n)]
    z256 = np.ascontiguousarray(
        (Z_l[:, :, D] / np.float32(255.0)).reshape(B, NT, 128).transpose(0, 2, 1)
    ).astype(bf)                                                           # [B, 128, 16]
    wvb = np.ascontiguousarray(
        np.broadcast_to(Wv[D, :] / np.sqrt(np.float32(DK)), (128, DV))
    ).astype(bf)                                                           # [128, 257]
    Wv2 = (Wv[:D, :] * scale).astype(bf)
    Wqb = Wq.astype(bf)
    Wkb = Wk.astype(bf)

    if "nc" not in _CACHE:
        _CACHE["nc"] = _build_nc()
    nc = _CACHE["nc"]

    in_maps = [
        {
            "m8T": M8T[b],
            "XT": XT[b],
            "ZT": ZT[b],
            "z256": z256[b],
            "wvb": wvb,
            "Wq": Wqb,
            "Wk": Wkb,
            "Wv2": Wv2,
        }
        for b in range(B)
    ]
    try:
        res = run_bass_kernel_spmd(nc, in_maps, core_ids=list(range(B)), trace=trace)
    except Exception:
        # A prior (profiled) run can leave an execution unit wedged; the failed
        # attempt clears it and a retry goes through.
        res = run_bass_kernel_spmd(nc, in_maps, core_ids=list(range(B)), trace=trace)
    _CACHE["last_res"] = res
    if trace:
        LAST_EXEC_NS = res.exec_time_ns
    out = np.stack([res.results[b]["out"] for b in range(B)], axis=0)
    return out


# revision 5
# speedup vs baseline: 1.0302x; 1.0302x over previous
"""Trainium2 Bass kernel for nn_AttentionHead_5583457485447 (sparse_attention).

Reference computation (per batch b):
    q = X @ Wq; k = X @ Wk                      # [N, DK]
    s = relu((q @ k.T) / sqrt(DK)) * M_mask     # [N, N]
    out = s @ Z @ Wv                            # [N, DV]

Strategy (8 NeuronCores, data-parallel over batch B=8, one batch per core):
  - Fold 1/sqrt(DK) into Wv (relu is positively homogeneous, rest is linear).
  - Mask quantized to uint8 (m8 = round(m*255)); the 1/255 is folded into
    Wv as well.  Halves mask HBM traffic; adds ~2e-3 rel error (budget 2e-2).
  - ZW = Z @ (Wv/(8*255)) computed on device; the v=256 contraction sliver
    (rank-1 term z256 (x) wv256) is NOT done with K=1 matmuls: z256/255 is
    appended as a 258th column to each zw tile, so the big C matmul
    accumulates u[n] = masked8 @ z256' for free, and a fused DVE
    scalar_tensor_tensor adds u[p]*wvb + psum during the PSUM->SBUF copy.
  - q/k projections run concurrently in PE column groups 0:64 / 64:128 of
    one pass (tile_position=(0,64) for k), giving stacked [qT; kT] tiles;
    the swapped [kT; qT] layout is produced by SBUF->SBUF DMA so the score
    matmuls can row-pack two K=64 m-chunks per PE pass (rows 0:64 / 64:128).
  - Scores computed directly in transposed [m, n] layout (lhsT = kT,
    rhs = qT): already the lhsT layout the second matmul needs.
  - relu+mask fused: DVE scalar_tensor_tensor max(s,0)*m8 from PSUM for most
    tiles; ACT relu + GpSimd multiply for the rest (engine balance).
  - Quarter-granularity software pipeline (n in 4 chunks of 512): C(q)
    overlaps B(q+2) elementwise + scores, masks stream in per quarter.
"""

import json
import os
import sys

import numpy as np

B, N, D, DK = 8, 2048, 256, 64
DV = D + 1  # 257
NT = N // 128  # 16 tiles along n and along m
PW = 512  # quarter width = scores matmul moving width
NQ = N // PW  # 4 quarters
QT = PW // 128  # 4 n-tiles per quarter

LAST_EXEC_NS = None
_CACHE = {}


# --------------------------------------------------------------------------
# Patch: this container's walrus build rejects instructions carrying more
# than one semaphore wait. Split excess waits onto same-engine NOPs at the
# serialized-BIR level (generic, covers Tile's drains and compute ops).
# --------------------------------------------------------------------------
def _split_waits_in_bir(bir_json: bytes) -> bytes:
    bir = json.loads(bir_json)
    changed = False
    for fn in bir.get("functions", []):
        for bb in fn.get("blocks", []):
            insts = bb.get("instructions", [])
            out = []
            for inst in insts:
                si = inst.get("sync_info")
                ow = (si or {}).get("on_wait") or []
                if len(ow) > 1:
                    changed = True
                    for i, w in enumerate(ow[:-1]):
                        out.append({
                            "debug": inst.get("debug", 0),
                            "engine": inst["engine"],
                            "ins": [],
                            "name": f"{inst['name']}-ws{i}",
                            "opcode": "NoOp",
                            "outs": [],
                            "sync_info": {"on_update": [], "on_wait": [w]},
                            "text_hint": "wait_split",
                        })
                    si["on_wait"] = [ow[-1]]
                out.append(inst)
            bb["instructions"] = out
    return json.dumps(bir).encode() if changed else bir_json


def _apply_bir_patch():
    import concourse.bass_utils as bass_utils
    import concourse.bass2jax as bass2jax

    orig = bass_utils.compile_bir_kernel
    if getattr(orig, "_wait_split_wrapped", False):
        return

    def wrapped(bir_json, tmpdir, neff_name="file.neff"):
        if isinstance(bir_json, str):
            bir_json = bir_json.encode()
        return orig(_split_waits_in_bir(bir_json), tmpdir, neff_name=neff_name)

    wrapped._wait_split_wrapped = True
    bass_utils.compile_bir_kernel = wrapped
    bass2jax.compile_bir_kernel = wrapped


# --------------------------------------------------------------------------
# Optional NTFF profiling hook for axon (exec-time measurement).
# Only used when KERNEL_TRACE=1; missing in this image's antenv.
# --------------------------------------------------------------------------
def _install_profile_shim():
    import types, ctypes, contextlib

    if "antenv.axon_hooks" in sys.modules:
        return
    so_path = "/opt/axon/libaxon_pjrt.so"
    if not os.path.exists(so_path):
        return
    lib = ctypes.CDLL(so_path)
    if not hasattr(lib, "axon_start_nrt_profile"):
        return
    lib.axon_start_nrt_profile.argtypes = [ctypes.POINTER(ctypes.c_int64), ctypes.c_size_t]
    lib.axon_start_nrt_profile.restype = ctypes.c_int64
    lib.axon_stop_nrt_profile.argtypes = [ctypes.c_char_p]
    lib.axon_stop_nrt_profile.restype = ctypes.c_int64

    @contextlib.contextmanager
    def _hook(output_dir, device_ids):
        import jax

        jax.devices()
        if device_ids:
            ids = (ctypes.c_int64 * len(device_ids))(*device_ids)
            rc = lib.axon_start_nrt_profile(ids, len(device_ids))
        else:
            rc = lib.axon_start_nrt_profile(None, 0)
        if rc != 0:
            raise RuntimeError(f"axon_start_nrt_profile rc={rc}")
        try:
            yield
        finally:
            n = lib.axon_stop_nrt_profile(str(output_dir).encode())
            print(f"profile: {n} file(s) written to {output_dir}", file=sys.stderr)

    mod = types.ModuleType("antenv.axon_hooks")
    mod.get_axon_ntff_profile_hook = lambda: _hook
    sys.modules["antenv.axon_hooks"] = mod


# --------------------------------------------------------------------------
# Device program (identical for all 8 cores; one batch per core)
# --------------------------------------------------------------------------
def _build_nc():
    import concourse.bass as bass
    import concourse.mybir as mybir
    import concourse.tile as tile

    f32 = mybir.dt.float32
    bf16 = mybir.dt.bfloat16
    u8 = mybir.dt.uint8
    Alu = mybir.AluOpType
    Act = mybir.ActivationFunctionType

    nc = bass.Bass("TRN2", debug=False)

    d_m8T = nc.dram_tensor("m8T", [N, N], u8, kind="ExternalInput")
    d_XT = nc.dram_tensor("XT", [D, N], bf16, kind="ExternalInput")
    d_ZT = nc.dram_tensor("ZT", [D, N], bf16, kind="ExternalInput")  # rows 0:256 of Z^T
    d_z256 = nc.dram_tensor("z256", [128, NT], bf16, kind="ExternalInput")
    d_wvb = nc.dram_tensor("wvb", [128, DV], bf16, kind="ExternalInput")
    d_Wq = nc.dram_tensor("Wq", [D, DK], bf16, kind="ExternalInput")
    d_Wk = nc.dram_tensor("Wk", [D, DK], bf16, kind="ExternalInput")
    d_Wv2 = nc.dram_tensor("Wv2", [D, DV], bf16, kind="ExternalInput")
    d_out = nc.dram_tensor("out", [N, DV], f32, kind="ExternalOutput")

    with tile.TileContext(nc) as tc:
        with (
            tc.tile_pool(name="prep", bufs=1) as prep,       # XT/ZT staging
            tc.tile_pool(name="wts", bufs=1) as wts,         # weights + QK tiles
            tc.tile_pool(name="maskp", bufs=1) as maskp,     # all mask tiles resident
            tc.tile_pool(name="maskedp", bufs=1) as maskedp, # all masked tiles resident
            tc.tile_pool(name="zwp", bufs=1) as zwp,         # bf16 ZW tiles (258 wide)
            tc.tile_pool(name="outp", bufs=3) as outp,       # out staging
            tc.tile_pool(name="rlp", bufs=4) as rlp,         # relu staging (ACT path)
            tc.tile_pool(name="psS", bufs=5, space="PSUM") as psS,   # proj/zw/scores
            tc.tile_pool(name="psC", bufs=3, space="PSUM") as psC,   # C groups
        ):
            # ---- input DMAs, in priority order ----
            wq_sb = [wts.tile([128, DK], bf16, tag=f"wq{c}", name=f"wq{c}") for c in range(2)]
            wk_sb = [wts.tile([128, DK], bf16, tag=f"wk{c}", name=f"wk{c}") for c in range(2)]
            for c in range(2):
                nc.gpsimd.dma_start(wq_sb[c][:], d_Wq.ap()[c * 128:(c + 1) * 128, :])
                nc.gpsimd.dma_start(wk_sb[c][:], d_Wk.ap()[c * 128:(c + 1) * 128, :])
            xt_sb = [[prep.tile([128, PW], bf16, tag=f"xt{c}_{g}", name=f"xt{c}_{g}")
                      for g in range(NQ)] for c in range(2)]
            for g in range(NQ):
                for c in range(2):
                    nc.sync.dma_start(
                        xt_sb[c][g][:],
                        d_XT.ap()[c * 128:(c + 1) * 128, g * PW:(g + 1) * PW],
                    )
            wv2_sb = [wts.tile([128, DV], bf16, tag=f"wv{i}", name=f"wv{i}") for i in range(2)]
            for i in range(2):
                nc.scalar.dma_start(wv2_sb[i][:], d_Wv2.ap()[i * 128:(i + 1) * 128, :])
            z256_sb = wts.tile([128, NT], bf16, tag="z256", name="z256")
            nc.scalar.dma_start(z256_sb[:], d_z256.ap()[:, :])
            wvb_sb = wts.tile([128, DV], bf16, tag="wvb", name="wvb")
            nc.scalar.dma_start(wvb_sb[:], d_wvb.ap()[:, :])
            # mask tiles: [128, PW] u8, all resident; q0 masks go before ZT,
            # ZT before q1 masks (zw needed when C(0) starts), rest after.
            mk = {}

            def emit_mask_q(q):
                for mt in range(NT):
                    t = maskp.tile([128, PW], u8, tag=f"mk{q}_{mt}", name=f"mk{q}_{mt}")
                    mk[(q, mt)] = t
                    nc.sync.dma_start(
                        t[:],
                        d_m8T.ap()[mt * 128:(mt + 1) * 128, q * PW:(q + 1) * PW],
                    )

            emit_mask_q(0)
            zt_sb = [prep.tile([128, N], bf16, tag=f"zt{i}", name=f"zt{i}") for i in range(2)]
            for g in range(NQ):
                for i in range(2):
                    nc.scalar.dma_start(
                        zt_sb[i][:, g * PW:(g + 1) * PW],
                        d_ZT.ap()[i * 128:(i + 1) * 128, g * PW:(g + 1) * PW],
                    )
            for q in range(1, NQ):
                emit_mask_q(q)

            # ---- PE warm-up: a few dummy matmuls engage the HAM clock
            # un-throttle while the first XT chunks stream in. ----
            wu = wts.tile([128, PW], bf16, tag="wu", name="wu")
            nc.gpsimd.memset(wu[:], 0.0)
            for w in range(5):
                pw = psS.tile([128, PW], f32, tag="psS", name=f"psw{w}")
                nc.tensor.matmul(pw[:], wu[:, :128], wu[:], start=True, stop=True)

            # ---- projections: q into PE cols 0:64, k into cols 64:128 ----
            # psQK = [qT_g (64 rows); kT_g (64 rows)] per column chunk g.
            qk_a = [wts.tile([128, PW], bf16, tag=f"qka{g}", name=f"qka{g}") for g in range(NQ)]
            qk_b = [wts.tile([128, PW], bf16, tag=f"qkb{g}", name=f"qkb{g}") for g in range(NQ)]
            for g in range(NQ):
                ps = psS.tile([128, PW], f32, tag="psS", name=f"psqk{g}")
                for c in range(2):
                    nc.tensor.matmul(
                        ps[0:DK, :], wq_sb[c][:], xt_sb[c][g][:],
                        start=(c == 0), stop=(c == 1),
                    )
                for c in range(2):
                    nc.tensor.matmul(
                        ps[DK:128, :], wk_sb[c][:], xt_sb[c][g][:],
                        start=(c == 0), stop=(c == 1),
                        tile_position=(0, DK),
                    )
                nc.scalar.activation(qk_a[g][:], ps[:], Act.Copy)
                # swapped layout for the row-packed score matmuls
                nc.scalar.dma_start(qk_b[g][DK:128, :], qk_a[g][0:DK, :])
                nc.scalar.dma_start(qk_b[g][0:DK, :], qk_a[g][DK:128, :])

            # ---- ZW = Z[:, :256] @ Wv2, with z256/255 appended as col 257 ----
            zw_sb = [None] * NT

            def emit_zw(mt):
                ps = psS.tile([128, DV], f32, tag="psS", name=f"pszw{mt}")
                for i in range(2):
                    nc.tensor.matmul(
                        ps[:],
                        zt_sb[i][:, mt * 128:(mt + 1) * 128],
                        wv2_sb[i][:],
                        start=(i == 0), stop=(i == 1),
                    )
                zw = zwp.tile([128, DV + 1], bf16, tag=f"zw{mt}", name=f"zw{mt}")
                nc.scalar.activation(zw[:, 0:DV], ps[:], Act.Copy)
                nc.vector.tensor_copy(zw[:, DV:DV + 1], z256_sb[:, mt:mt + 1])
                zw_sb[mt] = zw

            # ---- B(q): scores + relu*mask for one n-quarter ----
            masked_sb = {}
            ew = 0  # elementwise rotation counter

            def emit_b_pair(q, pr):
                nonlocal ew
                mts = (2 * pr, 2 * pr + 1)
                for j, mt in enumerate(mts):
                    gk, kcol = divmod(mt * 128, PW)
                    ro = DK * j
                    # j=0: lhsT = kT from qk_b rows 0:64; rhs = qT from qk_a rows 0:64
                    # j=1: lhsT = kT from qk_a rows 64:128; rhs = qT from qk_b rows 64:128
                    lhsT = (qk_b if j == 0 else qk_a)[gk][ro:ro + DK, kcol:kcol + 128]
                    rhs = (qk_a if j == 0 else qk_b)[q][ro:ro + DK, :]
                    ps = psS.tile([128, PW], f32, tag="psS", name=f"pss{q}_{mt}")
                    nc.tensor.matmul(ps[:], lhsT, rhs, start=True, stop=True)
                    md = maskedp.tile([128, PW], bf16, tag=f"md{q}_{mt}", name=f"md{q}_{mt}")
                    masked_sb[(q, mt)] = md
                    if ew % 8 in (2, 5, 7):
                        rl = rlp.tile([128, PW], bf16, tag="rl", name=f"rl{q}_{mt}")
                        nc.scalar.activation(rl[:], ps[:], Act.Relu)
                        nc.gpsimd.tensor_mul(md[:], rl[:], mk[(q, mt)][:])
                    else:
                        nc.vector.scalar_tensor_tensor(
                            md[:], ps[:], 0.0, mk[(q, mt)][:], Alu.max, Alu.mult,
                        )
                    ew += 1

            # ---- C(q, nt): out n-tile = sum_mt masked.T @ zw ----
            def emit_c_group(q, i):
                nt = q * QT + i
                ps = psC.tile([128, DV + 1], f32, tag="psC", name=f"psc{nt}")
                for mt in range(NT):
                    nc.tensor.matmul(
                        ps[:],
                        masked_sb[(q, mt)][:, i * 128:(i + 1) * 128],
                        zw_sb[mt][:],
                        start=(mt == 0), stop=(mt == NT - 1),
                    )
                ot = outp.tile([128, DV], f32, tag="out", name=f"ot{nt}")
                # out = wvb * u + main   (u = psum col 257)
                nc.vector.scalar_tensor_tensor(
                    ot[:], wvb_sb[:], ps[:, DV:DV + 1], ps[:, 0:DV],
                    Alu.mult, Alu.add,
                )
                nc.sync.dma_start(
                    d_out.ap()[nt * 128:(nt + 1) * 128, :], ot[:]
                )

            # ---- software pipeline over quarters ----
            # proj -> B(0) -> [ZW x B(1)] -> [C(0) x B(2)] -> [C(1) x B(3)]
            # -> C(2) -> C(3)
            for pr in range(NT // 2):
                emit_b_pair(0, pr)
            for pr in range(NT // 2):
                emit_zw(2 * pr)
                emit_zw(2 * pr + 1)
                emit_b_pair(1, pr)
            for q in (2, 3):
                for pr in range(NT // 2):
                    emit_b_pair(q, pr)
                    if pr % 2 == 1:
                        emit_c_group(q - 2, pr // 2)
            for q in (2, 3):
                for i in range(QT):
                    emit_c_group(q, i)

    return nc


def kernel(Z_l, X_l, M_mask, Wq, Wk, Wv):
    global LAST_EXEC_NS
    _apply_bir_patch()

    trace = os.environ.get("KERNEL_TRACE", "0") == "1"
    if trace:
        _install_profile_shim()

    from concourse.bass_utils import run_bass_kernel_spmd

    Z_l = np.asarray(Z_l, dtype=np.float32)
    X_l = np.asarray(X_l, dtype=np.float32)
    M_mask = np.asarray(M_mask, dtype=np.float32)
    Wq = np.asarray(Wq, dtype=np.float32)
    Wk = np.asarray(Wk, dtype=np.float32)
    Wv = np.asarray(Wv, dtype=np.float32)

    import ml_dtypes
    bf = ml_dtypes.bfloat16

    # Host-side layout prep (transpose + casts) + scale folds.
    scale = np.float32(1.0 / (np.sqrt(np.float32(DK)) * 255.0))
    XT = np.ascontiguousarray(X_l.transpose(0, 2, 1)).astype(bf)          # [B, D, N]
    ZT = np.ascontiguousarray(Z_l[:, :, :D].transpose(0, 2, 1)).astype(bf)  # [B, 256, N]
    M8T = np.ascontiguousarray(
        np.clip(np.round(M_mask * 255.0), 0, 255).astype(np.uint8).transpose(0, 2, 1)
    )                                                                      # [B, N(m), N(n)]
    z256 = np.ascontiguousarray(
        (Z_l[:, :, D] / np.float32(255.0)).reshape(B, NT, 128).transpose(0, 2, 1)
    ).astype(bf)                                                           # [B, 128, 16]
    wvb = np.ascontiguousarray(
        np.broadcast_to(Wv[D, :] / np.sqrt(np.float32(DK)), (128, DV))
    ).astype(bf)                                                           # [128, 257]
    Wv2 = (Wv[:D, :] * scale).astype(bf)
    Wqb = Wq.astype(bf)
    Wkb = Wk.astype(bf)

    if "nc" not in _CACHE:
        _CACHE["nc"] = _build_nc()
    nc = _CACHE["nc"]

    in_maps = [
        {
            "m8T": M8T[b],
            "XT": XT[b],
            "ZT": ZT[b],
            "z256": z256[b],
            "wvb": wvb,
            "Wq": Wqb,
            "Wk": Wkb,
            "Wv2": Wv2,
        }
        for b in range(B)
    ]
    try:
        res = run_bass_kernel_spmd(nc, in_maps, core_ids=list(range(B)), trace=trace)
    except Exception:
        # A prior (profiled) run can leave an execution unit wedged; the failed
        # attempt clears it and a retry goes through.
        res = run_bass_kernel_spmd(nc, in_maps, core_ids=list(range(B)), trace=trace)
    _CACHE["last_res"] = res
    if trace:
        LAST_EXEC_NS = res.exec_time_ns
    out = np.stack([res.results[b]["out"] for b in range(B)], axis=0)
    return out
